# revision 1
# baseline (speedup 1.0000x reference)
"""Trainium2 Bass kernel for a 2-layer GCN (FCGraphGNN) over 8 NeuronCores.

Math (matches reference):
  norm_e = dinv[src]*ew*dinv[dst] (self loops included), precomputed host-side
  h1 = relu(segsum_dst(norm * (x@W1)[src]) + b1)
  h2 = relu(segsum_dst(norm * (h1@W2)[src]) + b2)
  out = mean-pool-by-graph(h2) @ Wo + bo

Device strategy (v2 — ap_gather pipeline):
  - Edges sharded by dst across 8 cores; dst nodes packed into windows of
    <=64 consecutive nodes with [lo | hi] source-side slot capacities.
  - Per-layer source table lives in SBUF feature-major: [128, NVH] f32 with
    partitions 0-63 = 64 feats of the lo half nodes, 64-127 = hi half.
  - gpsimd.ap_gather pulls per-edge message columns M^T for one window per
    call (all 8 Q7 cores busy: 4 on lo slots, 4 on hi).
  - Per 128-slot tile: PE transpose -> slot-major M; matmul with
    host-precomputed S (norm one-hot, streamed bf16 from DRAM) accumulates
    the window's h^T [64 feat, 64 dst] in PSUM.
  - Window epilogue applies relu+bias on ACT, W2 transform (layer 1) or
    graph-pool accumulation (layer 2) on PE.
  - h1@W2 all-gathered across cores between layers; pooled partials
    all-reduced at the end.
"""

import os
import sys
import types

import numpy as np

sys.path.insert(0, "/opt/trn_rl_repo")


def _install_ntff_hook():
    """Best-effort: the container's antenv stub may lack axon_hooks, which
    run_bass_kernel_spmd imports under BASS_TRACE=1. Inject a shim wired to
    the libaxon NTFF profiler so tracing works instead of crashing."""
    if "antenv.axon_hooks" in sys.modules:
        return
    try:
        import antenv
    except ImportError:
        return
    try:
        import antenv.axon_hooks  # noqa: F401

        return
    except ImportError:
        pass
    mod = types.ModuleType("antenv.axon_hooks")
    mod._hook = None
    mod.set_axon_ntff_profile_hook = lambda h: setattr(mod, "_hook", h)
    mod.get_axon_ntff_profile_hook = lambda: mod._hook
    sys.modules["antenv.axon_hooks"] = mod
    antenv.axon_hooks = mod
    try:
        from trn_agent_boot.trn_boot import _ntff_profile_via_ctypes

        hook = _ntff_profile_via_ctypes("/opt/axon/libaxon_pjrt.so")
        if hook is not None:
            mod.set_axon_ntff_profile_hook(hook)
    except Exception:
        pass


_install_ntff_hook()

# ---------------------------------------------------------------- constants
N_NODES = 50000
N_EDGES = 3200000
N_GRAPHS = 50
IN_F = 5
HID = 64
OUT_F = 2
N_CORES = 8

SLOTS = 64          # dst nodes per window
T_SIDE = 16         # 128-slot tiles per (window, src-half)
CAP = T_SIDE * 128  # edge slots per (window, side)
SG = 52             # graph columns (50 graphs + 2 pad)
XCH = 4096          # xT chunk columns for the preamble


def _pack_host(x, edge_index, edge_attr, batch):
    """Index/layout preprocessing (numpy). Returns per-core input dicts plus
    the static plan."""
    import ml_dtypes

    src = np.asarray(edge_index[0], dtype=np.int64)
    dst = np.asarray(edge_index[1], dtype=np.int64)
    ew = np.asarray(edge_attr, dtype=np.float32).reshape(-1)
    loop = np.arange(N_NODES, dtype=np.int64)
    src = np.concatenate([src, loop]).astype(np.int64)
    dst = np.concatenate([dst, loop]).astype(np.int64)
    ew = np.concatenate([ew, np.ones(N_NODES, np.float32)])
    E = src.shape[0]

    # symmetric normalization, host-side (pure function of the inputs)
    deg = np.zeros(N_NODES, np.float64)
    np.add.at(deg, dst, ew.astype(np.float64))
    dinv = np.where(deg > 0, 1.0 / np.sqrt(np.maximum(deg, 1e-30)), 0.0)
    norm = (dinv[src] * ew * dinv[dst]).astype(np.float32)

    deg_cnt = np.bincount(dst, minlength=N_NODES).astype(np.int64)
    node_ptr = np.zeros(N_NODES + 1, np.int64)
    np.cumsum(deg_cnt, out=node_ptr[1:])
    order = np.argsort(dst, kind="stable")

    # core node boundaries balancing edge counts
    cum = node_ptr[1:]
    nb = [0]
    for c in range(1, N_CORES):
        nb.append(int(np.searchsorted(cum, c * E / N_CORES)))
    nb.append(N_NODES)
    nb = np.array(nb, np.int64)
    split_node = int(nb[4])  # src < split_node -> "lo" half of virtual space

    side_lo = src < split_node
    deg_lo = np.bincount(dst[side_lo], minlength=N_NODES).astype(np.int64)
    deg_hi = deg_cnt - deg_lo

    # window packing per core
    core_windows = []
    for c in range(N_CORES):
        wlist = []
        v = int(nb[c])
        end = int(nb[c + 1])
        while v < end:
            ws = v
            lo = hi = cnt = 0
            while (
                v < end
                and cnt < SLOTS
                and lo + deg_lo[v] <= CAP
                and hi + deg_hi[v] <= CAP
            ):
                lo += int(deg_lo[v])
                hi += int(deg_hi[v])
                cnt += 1
                v += 1
            wlist.append((ws, v))
        core_windows.append(wlist)

    NW = max(len(w) for w in core_windows)
    NW = (NW + 1) // 2 * 2  # even -> NVH multiple of 512
    assert NW <= 128, f"NW={NW} exceeds int16 index budget"
    NVC = NW * SLOTS
    NV = N_CORES * NVC
    NVH = NV // 2
    assert NVH <= 32768

    # vid map (node -> virtual id)
    node_vid = np.zeros(N_NODES, np.int32)
    for c in range(N_CORES):
        for w, (ws, we) in enumerate(core_windows[c]):
            node_vid[ws:we] = c * NVC + w * SLOTS + np.arange(we - ws, dtype=np.int32)

    # per-side dst-sorted edge lists + ptrs
    lo_edges = order[side_lo[order]]
    hi_edges = order[~side_lo[order]]
    lo_ptr = np.zeros(N_NODES + 1, np.int64)
    np.cumsum(deg_lo, out=lo_ptr[1:])
    hi_ptr = np.zeros(N_NODES + 1, np.int64)
    np.cumsum(deg_hi, out=hi_ptr[1:])

    vid_src = node_vid[src]
    IDXC = CAP // 16

    # xt in virtual layout (shared by all cores)
    xt_virt = np.zeros((IN_F, NV), np.float32)
    xt_virt[:, node_vid] = np.asarray(x, np.float32).T

    batch_i = np.asarray(batch, np.int64)
    cnt_g = np.bincount(batch_i, minlength=N_GRAPHS).astype(np.float32)
    inv_cnt = 1.0 / np.maximum(cnt_g, 1.0)

    def wrap16(a):  # [CAP] -> [16, CAP//16] with unwrapped[i] = w[i%16, i//16]
        return np.ascontiguousarray(a.reshape(IDXC, 16).T)

    per_core = []
    for c in range(N_CORES):
        wlist = core_windows[c]
        idxs = np.zeros((NW // 2, 128, IDXC), np.int16)
        S = np.zeros((NW, 128, 2 * T_SIDE, SLOTS), ml_dtypes.bfloat16)
        Sg = np.zeros((SLOTS, NW, SG), ml_dtypes.bfloat16)

        for w, (ws, we) in enumerate(wlist):
            for s, (edges, ptr, voff) in enumerate(
                ((lo_edges, lo_ptr, 0), (hi_edges, hi_ptr, NVH))
            ):
                ids = edges[ptr[ws] : ptr[we]]
                n = ids.shape[0]
                sl = np.zeros(CAP, np.int16)
                sl[:n] = (vid_src[ids] - voff).astype(np.int16)
                # call k covers windows (2k, 2k+1): 32-chan group per
                # (window, side), idx list replicated on its 2 cores
                base = (w % 2) * 64 + 32 * s
                idxs[w // 2, base : base + 32, :] = np.tile(wrap16(sl), (2, 1))
                # S[slot, tile, dstcol] = norm at filled slots
                tiles = np.arange(n) // 128
                rows = np.arange(n) % 128
                S[w, rows, s * T_SIDE + tiles, dst[ids] - ws] = norm[ids].astype(
                    ml_dtypes.bfloat16
                )
            nloc = we - ws
            g = batch_i[ws:we]
            Sg[np.arange(nloc), w, g] = inv_cnt[g].astype(ml_dtypes.bfloat16)

        per_core.append(
            dict(
                idxs=idxs,
                smat=np.ascontiguousarray(S.reshape(NW, 128, 2 * T_SIDE * SLOTS)),
                sg=np.ascontiguousarray(Sg.reshape(SLOTS, NW * SG)),
            )
        )

    plan = dict(NW=NW, NVC=NVC, NV=NV, NVH=NVH, IDXC=IDXC)
    return per_core, plan, xt_virt


def _build_program(plan):
    import concourse.bacc as bacc
    import concourse.tile as tile
    from concourse import mybir

    f32 = mybir.dt.float32
    bf16 = mybir.dt.bfloat16
    i16 = mybir.dt.int16
    Alu = mybir.AluOpType
    Act = mybir.ActivationFunctionType

    NW = plan["NW"]; NVC = plan["NVC"]; NV = plan["NV"]; NVH = plan["NVH"]
    IDXC = plan["IDXC"]
    NT = 2 * T_SIDE

    nc = bacc.Bacc("TRN2", target_bir_lowering=False, debug=False,
                   num_devices=N_CORES)

    xt = nc.declare_dram_parameter("xt", [IN_F, NV], f32, isOutput=False)
    w1e = nc.declare_dram_parameter("w1e", [IN_F, 32], f32, isOutput=False)
    w1o = nc.declare_dram_parameter("w1o", [IN_F, 32], f32, isOutput=False)
    w2e = nc.declare_dram_parameter("w2e", [HID, 32], bf16, isOutput=False)
    w2o = nc.declare_dram_parameter("w2o", [HID, 32], bf16, isOutput=False)
    wo = nc.declare_dram_parameter("wo", [HID, OUT_F], f32, isOutput=False)
    b1 = nc.declare_dram_parameter("b1", [HID, 1], f32, isOutput=False)
    b2 = nc.declare_dram_parameter("b2", [HID, 1], f32, isOutput=False)
    bo = nc.declare_dram_parameter("bo", [SG, OUT_F], f32, isOutput=False)
    idxs = nc.declare_dram_parameter("idxs", [NW // 2, 128, IDXC], i16,
                                     isOutput=False)
    smat = nc.declare_dram_parameter("smat", [NW, 128, NT * SLOTS], bf16,
                                     isOutput=False)
    sgp = nc.declare_dram_parameter("sg", [SLOTS, NW * SG], bf16, isOutput=False)
    identp = nc.declare_dram_parameter("ident", [128, 128], f32, isOutput=False)
    identbp = nc.declare_dram_parameter("identb", [SLOTS, SLOTS], bf16,
                                        isOutput=False)
    out = nc.declare_dram_parameter("out", [N_GRAPHS, OUT_F], f32, isOutput=True)
    chain_in = nc.declare_dram_parameter("chain", [1, 4], f32, isOutput=False)
    chain_out = nc.declare_dram_parameter("chain_out", [1, 4], f32, isOutput=True)

    groups = [list(range(N_CORES))]

    with tile.TileContext(nc) as tc:
        with (
            tc.tile_pool(name="dram", bufs=1, space="DRAM") as dram,
            tc.tile_pool(name="const", bufs=1) as cpool,
            tc.tile_pool(name="table", bufs=1) as tpool,
        ):
            h12loc = dram.tile([32, NVC, 2], bf16, tag="h12loc")
            h12glob = dram.tile([N_CORES, 32, NVC, 2], bf16, tag="h12glob")
            pool_in_d = dram.tile([HID, SG], f32, tag="poolin")
            pool_out_d = dram.tile([HID, SG], f32, tag="poolout")

            # ---- constants
            w1es = cpool.tile([IN_F, 32], f32, tag="w1es")
            nc.sync.dma_start(w1es[:], w1e[:])
            w1os = cpool.tile([IN_F, 32], f32, tag="w1os")
            nc.sync.dma_start(w1os[:], w1o[:])
            w2es = cpool.tile([HID, 32], bf16, tag="w2es")
            nc.sync.dma_start(w2es[:], w2e[:])
            w2os = cpool.tile([HID, 32], bf16, tag="w2os")
            nc.sync.dma_start(w2os[:], w2o[:])
            wos = cpool.tile([HID, OUT_F], f32, tag="wos")
            nc.sync.dma_start(wos[:], wo[:])
            b1s = cpool.tile([HID, 1], f32, tag="b1s")
            nc.sync.dma_start(b1s[:], b1[:])
            b2s = cpool.tile([HID, 1], f32, tag="b2s")
            nc.sync.dma_start(b2s[:], b2[:])
            bos = cpool.tile([SG, OUT_F], f32, tag="bos")
            nc.sync.dma_start(bos[:], bo[:])
            sgs = cpool.tile([SLOTS, NW * SG], bf16, tag="sgs")
            nc.sync.dma_start(sgs[:], sgp[:])
            ident = cpool.tile([128, 128], f32, tag="ident")
            nc.sync.dma_start(ident[:], identp[:])
            identb = cpool.tile([SLOTS, SLOTS], bf16, tag="identb")
            nc.sync.dma_start(identb[:], identbp[:])

            # packed feature-pair table: chan c = feats (2c, 2c+1) of node n.
            # chans 0-31 lo half, 32-63 hi half, 64-127 replica.
            tbl = tpool.tile([128, NVH, 2], bf16, tag="tbl")

            # ---- preamble: pack (x @ W1)^T for all NV nodes into tbl
            NBLK = NVH // 512
            XB = (NV // 512) // 6 if (NV // 512) % 6 == 0 else 1
            with (
                tc.tile_pool(name="xchunk", bufs=2) as xpool,
                tc.tile_pool(name="preps", bufs=3, space="PSUM") as prepsum,
            ):
                xts_c = None
                for b in range(NV // 512):
                    if b % XB == 0:
                        xts_c = xpool.tile([IN_F, XB * 512], f32, tag="xts")
                        nc.sync.dma_start(
                            xts_c[:], xt[:, b * 512 : (b + XB) * 512]
                        )
                    xts = xts_c[:, (b % XB) * 512 : (b % XB) * 512 + 512]
                    poff = 0 if b < NBLK else 32
                    col = (b % NBLK) * 512
                    ps = prepsum.tile([128, 512], f32, tag="preps")
                    ps2 = prepsum.tile([128, 512], f32, tag="preps2")
                    for rep in (0, 64):
                        o = poff + rep
                        nc.tensor.matmul(
                            out=ps[o : o + 32, :], lhsT=w1es[:], rhs=xts,
                            start=True, stop=True, tile_position=(0, o),
                        )
                        nc.tensor.matmul(
                            out=ps2[o : o + 32, :], lhsT=w1os[:], rhs=xts,
                            start=True, stop=True, tile_position=(0, o),
                        )
                    for rep in (0, 64):
                        o = poff + rep
                        if rep == 0:
                            nc.scalar.activation(
                                tbl[o : o + 32, col : col + 512, 0],
                                ps[o : o + 32, :], Act.Copy,
                            )
                            nc.scalar.activation(
                                tbl[o : o + 32, col : col + 512, 1],
                                ps2[o : o + 32, :], Act.Copy,
                            )
                        else:
                            nc.vector.tensor_copy(
                                tbl[o : o + 32, col : col + 512, 0],
                                ps[o : o + 32, :],
                            )
                            nc.vector.tensor_copy(
                                tbl[o : o + 32, col : col + 512, 1],
                                ps2[o : o + 32, :],
                            )

            KCPY = int(os.environ.get("KCPY", "2"))

            # ---- per-layer pass over window-pair gather calls
            def layer(l):
                with (
                    tc.tile_pool(name=f"idx{l}", bufs=4) as ipool,
                    tc.tile_pool(name=f"mt{l}", bufs=3) as mpool,
                    tc.tile_pool(name=f"sw{l}", bufs=4) as spool,
                    tc.tile_pool(name=f"tp{l}", bufs=3, space="PSUM") as tppool,
                    tc.tile_pool(name=f"ms{l}", bufs=6) as mspool,
                    tc.tile_pool(name=f"acc{l}", bufs=3, space="PSUM") as apool,
                    tc.tile_pool(name=f"epi{l}", bufs=3) as epool,
                    tc.tile_pool(name=f"eps{l}", bufs=1, space="PSUM") as eppool,
                    tc.tile_pool(name=f"pl{l}", bufs=1, space="PSUM") as plpool,
                ):
                    if l == 2:
                        pool_ps = plpool.tile([HID, SG], f32, tag="poolps")

                    def epilogue(l, w, acc):
                        if l == 1:
                            hb = epool.tile([HID, SLOTS], bf16, tag="hb")
                            nc.scalar.activation(hb[:], acc[:], Act.Relu,
                                                 bias=b1s[:])
                            pse = eppool.tile([32, 2, SLOTS], f32, tag="pse")
                            nc.tensor.matmul(out=pse[:, 0, :], lhsT=w2es[:],
                                             rhs=hb[:], start=True, stop=True)
                            nc.tensor.matmul(out=pse[:, 1, :], lhsT=w2os[:],
                                             rhs=hb[:], start=True, stop=True)
                            stg = epool.tile([32, SLOTS, 2], bf16, tag="stg")
                            nc.scalar.activation(stg[:, :, 0], pse[:, 0, :],
                                                 Act.Copy)
                            nc.vector.tensor_copy(stg[:, :, 1], pse[:, 1, :])
                            nc.scalar.dma_start(
                                h12loc[:, w * SLOTS : (w + 1) * SLOTS, :],
                                stg[:],
                            )
                        else:
                            hb = epool.tile([HID, SLOTS], bf16, tag="hb2")
                            nc.scalar.activation(hb[:], acc[:], Act.Relu,
                                                 bias=b2s[:])
                            tp2 = eppool.tile([SLOTS, HID], bf16, tag="tp2b")
                            nc.tensor.transpose(tp2[:], hb[:], identb[:])
                            h2n = epool.tile([SLOTS, HID], bf16, tag="h2n")
                            nc.scalar.activation(h2n[:], tp2[:], Act.Copy)
                            nc.tensor.matmul(
                                out=pool_ps[:], lhsT=h2n[:],
                                rhs=sgs[:, w * SG : (w + 1) * SG],
                                start=(w == 0), stop=(w == NW - 1),
                            )

                    for c2 in range(NW // 2):
                        wA, wB = 2 * c2, 2 * c2 + 1
                        idxt = ipool.tile([128, IDXC], i16, tag="idxt")
                        nc.sync.dma_start(idxt[:], idxs[c2])
                        swA = spool.tile([128, NT, SLOTS], bf16, tag="swA")
                        nc.sync.dma_start(
                            swA[:], smat[wA].rearrange("p (t s) -> p t s", s=SLOTS)
                        )
                        swB = spool.tile([128, NT, SLOTS], bf16, tag="swB")
                        nc.sync.dma_start(
                            swB[:], smat[wB].rearrange("p (t s) -> p t s", s=SLOTS)
                        )
                        mt = mpool.tile([128, CAP], f32, tag="mt")
                        nc.gpsimd.ap_gather(
                            mt[:].bitcast(bf16), tbl[:], idxt[:],
                            channels=128, num_elems=NVH, d=2, num_idxs=CAP,
                        )
                        accA = apool.tile([HID, SLOTS], f32, tag="acc")
                        accB = apool.tile([HID, SLOTS], f32, tag="acc")
                        G = 4
                        for t0 in range(0, T_SIDE, G):
                            ts = list(range(t0, min(t0 + G, T_SIDE)))
                            tps, mss = [], []
                            for ti in ts:
                                # u32 transpose: [128 chans, 128 slots] ->
                                # [128 slots, 128 chans]; bf16 pair order kept
                                tp = tppool.tile([128, 128], f32, tag="tp")
                                nc.tensor.transpose(
                                    tp[:], mt[:, ti * 128 : ti * 128 + 128],
                                    ident[:],
                                )
                                tps.append(tp)
                            for i, ti in enumerate(ts):
                                ms = mspool.tile([128, 128], f32, tag="ms")
                                if KCPY and i % KCPY == 0:
                                    nc.vector.tensor_copy(ms[:], tps[i][:])
                                else:
                                    nc.scalar.activation(ms[:], tps[i][:],
                                                         Act.Copy)
                                mss.append(ms)
                            for i, ti in enumerate(ts):
                                ms = mss[i]
                                for q, (accq, swq) in enumerate(
                                    ((accA, swA), (accB, swB))
                                ):
                                    nc.tensor.matmul(
                                        out=accq[:],
                                        lhsT=ms[:, 64 * q : 64 * q + 32]
                                        .bitcast(bf16),
                                        rhs=swq[:, ti, :],
                                        start=(ti == 0), stop=False,
                                    )
                                    nc.tensor.matmul(
                                        out=accq[:],
                                        lhsT=ms[:, 64 * q + 32 : 64 * q + 64]
                                        .bitcast(bf16),
                                        rhs=swq[:, T_SIDE + ti, :],
                                        start=False, stop=(ti == T_SIDE - 1),
                                    )
                        epilogue(l, wA, accA)
                        epilogue(l, wB, accB)
                    if l == 2:
                        pst = epool.tile([HID, SG], f32, tag="pst")
                        nc.vector.tensor_copy(pst[:], pool_ps[:])
                        nc.scalar.dma_start(pool_in_d[:], pst[:])

            layer(1)

            # all-gather packed h1@W2, reload the table (incl. replica)
            nc.gpsimd.collective_compute(
                "AllGather", mybir.AluOpType.bypass, replica_groups=groups,
                ins=[h12loc[:].rearrange("a b j -> (a b j)")],
                outs=[h12glob[:].rearrange("r a b j -> (r a b j)")],
            )
            for r in range(N_CORES):
                poff = 0 if r < 4 else 32
                rr = r % 4
                for rep in (0, 64):
                    eng = nc.sync if rep == 0 else nc.scalar
                    eng.dma_start(
                        tbl[poff + rep : poff + rep + 32,
                            rr * NVC : (rr + 1) * NVC, :],
                        h12glob[r],
                    )

            layer(2)

            # ---- pooled partials -> all-reduce -> final linear
            nc.gpsimd.collective_compute(
                "AllReduce", mybir.AluOpType.add, replica_groups=groups,
                ins=[pool_in_d[:]], outs=[pool_out_d[:]],
            )
            with (
                tc.tile_pool(name="fin", bufs=1) as fpool,
                tc.tile_pool(name="finps", bufs=1, space="PSUM") as fpsum,
            ):
                pr = fpool.tile([HID, SG], f32, tag="pr")
                nc.sync.dma_start(pr[:], pool_out_d[:])
                pso = fpsum.tile([SG, OUT_F], f32, tag="pso")
                nc.tensor.matmul(out=pso[:], lhsT=pr[:], rhs=wos[:],
                                 start=True, stop=True)
                osb = fpool.tile([SG, OUT_F], f32, tag="osb")
                nc.vector.tensor_tensor(out=osb[:], in0=pso[:], in1=bos[:],
                                        op=Alu.add)
                nc.sync.dma_start(out[:], osb[0:N_GRAPHS, :])
                chs = fpool.tile([1, 4], f32, tag="chs")
                nc.sync.dma_start(chs[:], chain_in[:])
                nc.vector.tensor_scalar_add(chs[:], chs[:], 1.0)
                nc.sync.dma_start(chain_out[:], chs[:])

    nc.compile()
    return nc


def kernel(x, edge_index, edge_attr, batch, W1, b1, W2, b2, Wo, bo, **_):
    per_core, plan, xt_virt = _pack_host(x, edge_index, edge_attr, batch)
    nc = _build_program(plan)

    bo_t = np.zeros((SG, OUT_F), np.float32)
    bo_t[:N_GRAPHS, :] = 0.0
    bo_t[:, :] = np.tile(np.asarray(bo, np.float32).reshape(1, -1), (SG, 1))

    import ml_dtypes

    W1f = np.asarray(W1, np.float32)
    W2f = np.asarray(W2, np.float32)
    common = dict(
        chain=np.zeros((1, 4), np.float32),
        xt=xt_virt,
        ident=np.eye(128, dtype=np.float32),
        identb=np.eye(SLOTS, dtype=np.float32).astype(ml_dtypes.bfloat16),
        w1e=np.ascontiguousarray(W1f[:, 0::2]),
        w1o=np.ascontiguousarray(W1f[:, 1::2]),
        w2e=np.ascontiguousarray(W2f[:, 0::2]).astype(ml_dtypes.bfloat16),
        w2o=np.ascontiguousarray(W2f[:, 1::2]).astype(ml_dtypes.bfloat16),
        wo=np.asarray(Wo, np.float32),
        b1=np.asarray(b1, np.float32).reshape(HID, 1),
        b2=np.asarray(b2, np.float32).reshape(HID, 1),
        bo=bo_t,
    )
    in_maps = []
    for c in range(N_CORES):
        m = dict(common)
        m.update(per_core[c])
        in_maps.append(m)

    from concourse.bass_utils import run_bass_kernel_spmd

    res = run_bass_kernel_spmd(nc, in_maps, list(range(N_CORES)))
    out = res.results[0]["out"]
    kernel.last_exec_time_ns = res.exec_time_ns
    kernel.last_results = res.results
    return np.asarray(out, np.float32)


kernel.last_exec_time_ns = None

